# revision 11
# baseline (speedup 1.0000x reference)
"""Trainium2 Bass kernel for 2D erosion (3x3 sliding-window min) on
x: (8, 4, 1024, 1024) f32, padded with +1e9 at the borders (pad never wins).

Strategy: pure data parallel over the 32 (b, c) images -> 4 images per core.
Compute runs in bfloat16: min() commutes exactly with monotone rounding, so
the result equals bf16(true_min) (rel err <= 2^-9, far under the 2e-2 gate)
while HBM traffic and 16-bit DVE cycles both halve.

The per-core DRAM input is laid out with one 1e9 pad row between/around
images (shape (4*(1024+1)+1, 1024)) so every halo access is affine.

Per image, one SBUF tile [128 partitions x 8192]: partition p holds image
rows 8p..8p+7 concatenated along the free dim. The separable 3-tap min:
  - H(vertical) pass on DVE: all ops are contiguous unit-stride bf16 ->
    2x perf mode. Row-pair sharing s[k]=min(x[2k],x[2k+1]) then combines;
    boundary rows use a [128, 2048] halo tile holding DRAM rows 8p-1 and
    8p+8 (pad rows give border semantics).
  - W(horizontal) pass: the pair ops are stride-2 (no 2x mode on DVE), so
    they are split across engines: GpSimd (stride-insensitive) takes the
    pair-min sw[j]=min(v[2j],v[2j+1]) and the even combine, DVE takes the
    odd combine, and ACT fixes each image row's first/last column (window
    shrinks to 2 taps there = sw value).
Output is written in-place into the input tile and DMA'd out on the ACT
HWDGE ring (loads go on the SP ring, so they don't queue behind stores).
"""

import ml_dtypes
import numpy as np

import concourse.bass as bass
import concourse.bacc as bacc
import concourse.mybir as mybir
from concourse.tile import TileContext
from concourse.bass_utils import run_bass_kernel_spmd

N_CORES = 8
B, C, H, W = 8, 4, 1024, 1024
IMGS = B * C // N_CORES  # images per core = 4
P = 128                  # SBUF partitions
R = H // P               # image rows per partition = 8
F = R * W                # free-dim elements per partition = 8192
PAD = 1.0e9
XROWS = IMGS * (H + 1) + 1  # padded per-core input rows
BF16 = mybir.dt.bfloat16
NP_BF16 = ml_dtypes.bfloat16
MIN = mybir.AluOpType.min

_NC_CACHE = {}


def _build_nc(reps=1):
    nc = bacc.Bacc()
    x = nc.dram_tensor("x", (XROWS, W), BF16, kind="ExternalInput")
    y = nc.dram_tensor("y", (IMGS, H, W), BF16, kind="ExternalOutput")

    with TileContext(nc) as tc:
        with (
            tc.tile_pool(name="xp", bufs=3) as xpool,
            tc.tile_pool(name="hp", bufs=2) as hpool,
            tc.tile_pool(name="sp", bufs=2) as spool,
            tc.tile_pool(name="vp", bufs=2) as vpool,
            tc.tile_pool(name="wp", bufs=2) as wpool,
        ):
            seq = [im for _ in range(reps) for im in range(IMGS)]
            for idx, i in enumerate(seq):
                first, last = idx == 0, idx == len(seq) - 1
                base = 1 + i * (H + 1)  # first row of image i in padded DRAM

                xt = xpool.tile([P, F], BF16)
                halo = hpool.tile([P, 2 * W], BF16)

                # main load: image rows are contiguous in DRAM
                xm = x[base : base + H, :].rearrange("(p r) w -> p (r w)", p=P)
                # halo: partition p gets DRAM rows base-1+8p and base+8+8p
                # (9 rows apart); p=0 low / p=127 high land on 1e9 pad rows.
                hsrc = bass.AP(x, (base - 1) * W, [[R * W, P], [9 * W, 2], [1, W]])
                hdst = halo.rearrange("p (s w) -> p s w", s=2)
                if first:
                    # pipeline ramp: land rows 0-2 first so the split H pass
                    # (below) starts ~3us before the rest of the image arrives
                    nc.sync.dma_start(out=xt[:, 0 : 3 * W], in_=xm[:, 0 : 3 * W])
                    nc.sync.dma_start(out=hdst, in_=hsrc)
                    nc.sync.dma_start(out=xt[:, 3 * W : 5 * W], in_=xm[:, 3 * W : 5 * W])
                    nc.sync.dma_start(out=xt[:, 5 * W : F], in_=xm[:, 5 * W : F])
                else:
                    nc.sync.dma_start(out=xt, in_=xm)
                    nc.sync.dma_start(out=hdst, in_=hsrc)

                xr = xt.rearrange("p (r w) -> p r w", r=R)
                s = spool.tile([P, (R // 2) * W], BF16)        # [128, 4096]
                sr = s.rearrange("p (r w) -> p r w", r=R // 2)
                v = vpool.tile([P, F], BF16)                   # vertical-min result
                vr = v.rearrange("p (r w) -> p r w", r=R)

                # ---- H pass (DVE, contiguous bf16 -> 2x mode) ----
                halo2 = halo.rearrange("p (s w) -> p s w", s=2)
                if first:
                    # three groups matching the split load: A needs rows 0-2
                    # + low halo, B rows 3-4, C rows 5-7 + high halo
                    nc.vector.tensor_tensor(
                        out=sr[:, 0:1, :], in0=xr[:, 0:1, :], in1=xr[:, 1:2, :], op=MIN
                    )
                    nc.vector.tensor_tensor(
                        out=vr[:, 1:2, :], in0=sr[:, 0:1, :], in1=xr[:, 2:3, :], op=MIN
                    )
                    nc.vector.tensor_tensor(
                        out=vr[:, 0:1, :], in0=halo2[:, 0:1, :], in1=sr[:, 0:1, :], op=MIN
                    )
                    nc.vector.tensor_tensor(
                        out=sr[:, 1:2, :], in0=xr[:, 2:3, :], in1=xr[:, 3:4, :], op=MIN
                    )
                    nc.vector.tensor_tensor(
                        out=vr[:, 2:3, :], in0=xr[:, 1:2, :], in1=sr[:, 1:2, :], op=MIN
                    )
                    nc.vector.tensor_tensor(
                        out=vr[:, 3:4, :], in0=sr[:, 1:2, :], in1=xr[:, 4:5, :], op=MIN
                    )
                    nc.vector.tensor_tensor(
                        out=sr[:, 2:4, :], in0=xr[:, 4:R:2, :], in1=xr[:, 5:R:2, :], op=MIN
                    )
                    nc.vector.tensor_tensor(
                        out=vr[:, 4:7:2, :], in0=xr[:, 3:6:2, :], in1=sr[:, 2:4, :], op=MIN
                    )
                    nc.vector.tensor_tensor(
                        out=vr[:, 5:6, :], in0=sr[:, 2:3, :], in1=xr[:, 6:7, :], op=MIN
                    )
                    nc.vector.tensor_tensor(
                        out=vr[:, 7:8, :], in0=sr[:, 3:4, :], in1=halo2[:, 1:2, :], op=MIN
                    )
                else:
                    nc.vector.tensor_tensor(
                        out=sr, in0=xr[:, 0:R:2, :], in1=xr[:, 1:R:2, :], op=MIN
                    )
                    nc.vector.tensor_tensor(
                        out=vr[:, 2:R:2, :],
                        in0=xr[:, 1 : R - 1 : 2, :],
                        in1=sr[:, 1 : R // 2, :],
                        op=MIN,
                    )
                    nc.vector.tensor_tensor(
                        out=vr[:, 1 : R - 1 : 2, :],
                        in0=sr[:, 0 : R // 2 - 1, :],
                        in1=xr[:, 2:R:2, :],
                        op=MIN,
                    )
                    # boundary rows {0, R-1}: halo is one tile/one DMA
                    nc.vector.tensor_tensor(
                        out=vr[:, 0 : R : R - 1, :],
                        in0=halo2,
                        in1=sr[:, 0 : R // 2 : R // 2 - 1, :],
                        op=MIN,
                    )

                # ---- W pass: o[j] = min(v[j-1], v[j], v[j+1]) within rows ----
                # stride-2 APs get no 2x mode on DVE (and the compiler rejects
                # TensorTensor on Pool/GpSimd), so these three run at 1x.
                sw = wpool.tile([P, F // 2], BF16)             # [128, 4096]
                nc.vector.tensor_tensor(
                    out=sw, in0=v[:, 0:F:2], in1=v[:, 1:F:2], op=MIN
                )
                xtr = xt.rearrange("p (r w) -> p r w", r=R)
                swr = sw.rearrange("p (r w) -> p r w", r=R)    # rows of 512
                ym = y[i].rearrange("(p r) w -> p (r w)", p=P)

                # For the last image, emit the combines/fixes/store in two
                # row-aligned halves so the final store overlaps the final
                # combines (trims the pipeline drain); other images emit one
                # full-range "half" and store on the ACT ring only.
                halves = (
                    [
                        (0, F // 2),
                        (F // 2, 3 * F // 4),
                        (3 * F // 4, 7 * F // 8),
                        (7 * F // 8, F),
                    ]
                    if last
                    else [(0, F)]
                )
                for a, b in halves:
                    ev0 = max(a, 2)                   # even cols j: ev0..b-2
                    ob = b if b < F else F - 1        # odd cols j: a+1..ob-1
                    n_e = (b - ev0) // 2
                    n_o = len(range(a + 1, ob, 2))
                    nc.vector.tensor_tensor(
                        out=xt[:, ev0:b:2],
                        in0=v[:, ev0 - 1 : ev0 - 1 + 2 * n_e : 2],
                        in1=sw[:, ev0 // 2 : b // 2],
                        op=MIN,
                    )
                    nc.vector.tensor_tensor(
                        out=xt[:, a + 1 : ob : 2],
                        in0=sw[:, a // 2 : a // 2 + n_o],
                        in1=v[:, a + 2 : a + 2 + 2 * n_o : 2],
                        op=MIN,
                    )
                    # per-row first/last column: window shrinks to 2 taps =
                    # sw value; tiny strided copies go to the idle ACT.
                    ra, rb = a // W, b // W
                    nc.scalar.copy(
                        out=xtr[:, ra:rb, 0:1], in_=swr[:, ra:rb, 0:1]
                    )
                    nc.scalar.copy(
                        out=xtr[:, ra:rb, W - 1 : W],
                        in_=swr[:, ra:rb, W // 2 - 1 : W // 2],
                    )
                    # store on the ACT HWDGE ring (parallel to SP loads); the
                    # very last half goes out on the now-idle SP ring instead
                    if last and a > 0:
                        nc.sync.dma_start(out=ym[:, a:b], in_=xt[:, a:b])
                    else:
                        nc.scalar.dma_start(out=ym[:, a:b], in_=xt[:, a:b])

    nc.finalize()
    return nc


def _get_nc(reps=1):
    if reps not in _NC_CACHE:
        _NC_CACHE[reps] = _build_nc(reps)
    return _NC_CACHE[reps]


def _pad_shard(shard):
    """(IMGS, H, W) bf16 -> (XROWS, W) with a 1e9 pad row between/around."""
    out = np.full((XROWS, W), PAD, dtype=NP_BF16)
    for i in range(IMGS):
        base = 1 + i * (H + 1)
        out[base : base + H] = shard[i]
    return out


def kernel(x: np.ndarray, _reps: int = 1):
    x = np.asarray(x)
    assert x.shape == (B, C, H, W)
    xs = np.ascontiguousarray(x.astype(NP_BF16, copy=False)).reshape(
        N_CORES, IMGS, H, W
    )
    nc = _get_nc(_reps)
    in_maps = [{"x": _pad_shard(xs[k])} for k in range(N_CORES)]
    res = run_bass_kernel_spmd(nc, in_maps, core_ids=list(range(N_CORES)))
    out = np.stack([r["y"] for r in res.results], axis=0)
    return out.reshape(B, C, H, W).astype(np.float32)
